# revision 24
# baseline (speedup 1.0000x reference)
"""TRN2 Bass kernel for nn_Attention_59270548685139.

Custom two-stage-normalized attention, B=8, N=1024, D=1024, H=8, DH=64.
Sharding: data-parallel over batch -- one batch element per NeuronCore (8 cores).

Math per batch element (matching the reference):
  q = x @ Wq, k = x @ Wk, v = x @ Wv          (split into 8 heads of 64)
  sim[i,j]  = (q_i . k_j) * DH**-0.5
  attn      = softmax over the QUERY dim i    -> E[i,j]/C[j], C[j] = sum_i E[i,j]
  attn      = attn / (sum_j attn + eps)       -> per-i scale 1/(R[i]+eps)
  out       = attn @ v ; y = out @ Wo + bo

Key structural points:
- All matmul operands are bf16 (weights cast after DMA, activations cast on
  the PSUM->SBUF drains); PSUM accumulation stays fp32.  bf16 halves PE
  input energy vs fp32r, relaxing the HAM duty-cycle clock gate.
- Scores are computed transposed (S^T[j,i]) so the softmax-over-queries
  reduction is fused into the ACT exp pass (accum_out -> C[j]).
- The key-dim renormalization folds into a per-partition scale of V
  (1/C[j] via GPSIMD normalize_recip, fp32 in/out only -- GPSIMD casts are
  not HW-safe).  V carries an appended ones column so the same op also
  emits 1/C, and the attn@v matmul then produces R[i] for free.
- Software-pipelined head loop tuned for the IN-ORDER engine queues: each
  head emits S-blocks 0-2, then the PREVIOUS head's U matmuls + reciprocal,
  then S-blocks 3-7, so the ACT exp chain (the per-head critical path)
  always has scores to chew while the PE runs U/filler work.  et/v2 tiles
  are double-buffered across heads to allow the overlap.
- Q/K projections for later head-pairs are PE filler inside the head loop.
  Head 7's filler is PASS 1 of the output projection (heads 0-5 + bias),
  so the tail only runs the small heads-6/7 pass after the last
  normalization chain, instead of the whole output projection.
"""

import numpy as np

import concourse.bass as bass
import concourse.tile as tile
from concourse import bacc, mybir
from concourse.bass_utils import run_bass_kernel_spmd
from concourse.masks import make_identity

FP32 = mybir.dt.float32
FP32R = mybir.dt.float32r
BF16 = mybir.dt.bfloat16

B, N, D = 8, 1024, 1024
H, DH = 8, 64
INNER = H * DH  # 512
SCALE = DH ** -0.5
EPS = 1e-7
P = 128
NCORES = 8

_NC_CACHE = None


def _build_nc():
    nc = bacc.Bacc("TRN2", target_bir_lowering=False, debug=False)

    x_d = nc.dram_tensor("x", [N, D], FP32, kind="ExternalInput")
    wq_d = nc.dram_tensor("Wq", [D, INNER], FP32, kind="ExternalInput")
    wk_d = nc.dram_tensor("Wk", [D, INNER], FP32, kind="ExternalInput")
    wv_d = nc.dram_tensor("Wv", [D, INNER], FP32, kind="ExternalInput")
    wo_d = nc.dram_tensor("Wo", [INNER, D], FP32, kind="ExternalInput")
    bo_d = nc.dram_tensor("bo", [D], FP32, kind="ExternalInput")
    y_d = nc.dram_tensor("y", [N, D], FP32, kind="ExternalOutput")

    DC = D // P       # 8 contraction chunks over D
    IC = INNER // P   # 4 chunks over INNER
    NB = N // P       # 8 seq blocks of 128

    with tile.TileContext(nc) as tc:
        # ---------------- pools (all persistent; no phase barriers) ---------
        const_pool = tc.alloc_tile_pool(name="const", bufs=1)
        qt_pool = tc.alloc_tile_pool(name="qt", bufs=1)
        kt_pool = tc.alloc_tile_pool(name="kt", bufs=1)
        v_pool = tc.alloc_tile_pool(name="v", bufs=1)
        ot_pool = tc.alloc_tile_pool(name="ot", bufs=1)
        xt_pool = tc.alloc_tile_pool(name="xt", bufs=1)
        wv_pool = tc.alloc_tile_pool(name="wv", bufs=1)
        w4_pool = tc.alloc_tile_pool(name="w4", bufs=4)
        stg_pool = tc.alloc_tile_pool(name="stg", bufs=2)
        # 6 x-tile buffers so phase A's DMA streams 3 seq-blocks ahead of the
        # transposes instead of ping-ponging (DMA latency was serializing)
        xn_pool = tc.alloc_tile_pool(name="xn", bufs=5)
        xb_pool = tc.alloc_tile_pool(name="xb", bufs=2)
        et_pool = tc.alloc_tile_pool(name="et", bufs=2)
        sm_pool = tc.alloc_tile_pool(name="sm", bufs=2)
        smb_pool = tc.alloc_tile_pool(name="smb", bufs=2)
        usb_pool = tc.alloc_tile_pool(name="usb", bufs=3)
        y1_pool = tc.alloc_tile_pool(name="y1", bufs=1)
        y_pool = tc.alloc_tile_pool(name="yp", bufs=2)
        ps_pool = tc.alloc_tile_pool(name="ps", bufs=2, space="PSUM")

        # ---------------- constants ----------------
        ident = const_pool.tile([P, P], FP32, tag="ident")
        make_identity(nc, ident[:])
        identb = const_pool.tile([P, P], BF16, tag="identb")
        nc.scalar.copy(identb[:], ident[:])
        ones_b = const_pool.tile([1, P], BF16, tag="ones_b")
        nc.vector.memset(ones_b[:], 1.0)
        # bo as [1, 2, 512] bf16 (free-dim block db = bo[db*512:(db+1)*512])
        bo_stg = const_pool.tile([1, 2, 512], FP32, tag="bo_stg")
        nc.scalar.dma_start(
            out=bo_stg[:],
            in_=bo_d.ap().rearrange("(a n) -> a n", a=2)[None, :, :],
        )
        bo_r = const_pool.tile([1, 2, 512], BF16, tag="bo_r")
        nc.vector.tensor_copy(bo_r[:], bo_stg[:])

        # ---------------- persistent intermediates ----------------
        qt = [qt_pool.tile([P, N], BF16, tag=f"qt{m}", name=f"qt{m}") for m in range(IC)]
        kt = [kt_pool.tile([P, N], BF16, tag=f"kt{m}", name=f"kt{m}") for m in range(IC)]
        # V with an appended ones column per head: vts[jb][:, h, DH] == 1.0,
        # so normalize_recip emits V/C and 1/C in a single op.
        vts = [v_pool.tile([P, H, DH + 1], FP32, tag=f"v{j}", name=f"v{j}")
               for j in range(NB)]
        ot = [ot_pool.tile([P, N], BF16, tag=f"ot{m}", name=f"ot{m}") for m in range(IC)]
        xt = [xt_pool.tile([P, N], BF16, tag=f"xt{c}", name=f"xt{c}") for c in range(DC)]
        y1 = [y1_pool.tile([P, N], BF16, tag=f"y1_{ib}", name=f"y1_{ib}")
              for ib in range(NB)]

        # quarter-tile weight loader: fp32 staging DMA; cast emitted
        # separately (late in phase A) so it never blocks x processing
        def load_qk_dma(key, wd, mb):
            stg = stg_pool.tile([P, DC, P], FP32, tag="stq", name=f"stq{key}{mb}")
            nc.scalar.dma_start(
                out=stg[:],
                in_=wd.ap()[:, mb * P:(mb + 1) * P]
                .rearrange("(c p) n -> p c n", p=P),
            )
            return stg

        def load_qk_cast(key, mb, stg):
            w4 = w4_pool.tile([P, DC, P], BF16, tag="w4", name=f"w4{key}{mb}")
            if key == "q":
                nc.vector.tensor_copy(w4[:], stg[:])
            else:
                nc.scalar.copy(w4[:], stg[:])
            return w4

        # ---------------- phase A: load x (halves), transpose to xt --------
        wv_b = wv_pool.tile([P, DC, INNER], BF16, tag="wvb")
        w4q = {}
        qk_stg = {}
        wv_stg = {}
        for ib in range(NB):
            if ib == 1:
                # first weight DMAs (scalar queue; x stays on the sync queue)
                qk_stg[("q", 0)] = load_qk_dma("q", wq_d, 0)
                qk_stg[("k", 0)] = load_qk_dma("k", wk_d, 0)
            if ib in (2, 3):
                # Wv in four quarter-chunks through the 4KB staging slots
                for qq in range(2):
                    cq = 2 * (ib - 2) + qq
                    stg = stg_pool.tile([P, 2, INNER], FP32, tag="stw",
                                        name=f"stv{cq}")
                    nc.scalar.dma_start(
                        out=stg[:],
                        in_=wv_d.ap()[cq * 2 * P:(cq + 1) * 2 * P, :]
                        .rearrange("(c p) n -> p c n", p=P),
                    )
                    wv_stg[cq] = stg
            if ib == 4:
                # ones columns for the V tiles (gpsimd is idle in phase A)
                for jb in range(NB):
                    nc.gpsimd.memset(vts[jb][:, :, DH:DH + 1], 1.0)
            if ib == 6:
                # weight casts, deferred so their DMA waits never block the
                # x casts/drains queued behind them on ACT/DVE
                w4q[("q", 0)] = load_qk_cast("q", 0, qk_stg.pop(("q", 0)))
                w4q[("k", 0)] = load_qk_cast("k", 0, qk_stg.pop(("k", 0)))
                nc.scalar.copy(wv_b[:, 0:2, :], wv_stg[0][:])
                nc.vector.tensor_copy(wv_b[:, 2:4, :], wv_stg[1][:])
            if ib == 7:
                nc.scalar.copy(wv_b[:, 4:6, :], wv_stg[2][:])
                nc.vector.tensor_copy(wv_b[:, 6:8, :], wv_stg[3][:])
            xb = xb_pool.tile([P, N], BF16, tag="xb", name=f"xb{ib}")
            for hh in range(2):
                xh = xn_pool.tile([P, 512], FP32, tag="xn", name=f"xn{ib}_{hh}")
                nc.sync.dma_start(
                    out=xh[:],
                    in_=x_d.ap()[ib * P:(ib + 1) * P, hh * 512:(hh + 1) * 512],
                )
                if hh == 0:
                    nc.scalar.copy(xb[:, 0:512], xh[:])
                else:
                    nc.vector.tensor_copy(xb[:, 512:1024], xh[:])
            # transpose in bf16 (1 cycle/row vs 2 for fp32); out is a bf16
            # view of the fp32 PSUM tile
            p_t = ps_pool.tile([P, N], FP32, tag="big", name=f"ptp{ib}", bufs=3)
            for c in range(DC):
                nc.tensor.transpose(
                    p_t[:, c * 64:(c + 1) * 64].bitcast(BF16),
                    xb[:, c * P:(c + 1) * P],
                    identb[:],
                )
            for c in range(DC):
                src_ap = p_t[:, c * 64:(c + 1) * 64].bitcast(BF16)
                if c % 2 == 0:
                    nc.scalar.copy(xt[c][:, ib * P:(ib + 1) * P], src_ap)
                else:
                    nc.vector.tensor_copy(xt[c][:, ib * P:(ib + 1) * P], src_ap)

        # ---------------- projections (head-pair 0 + V up front) -----------
        def emit_qk_proj(key, dst, mb, drain=None):
            w4 = w4q.pop((key, mb))
            p_t = ps_pool.tile([P, N], FP32, tag="big", name=f"pp{key}{mb}", bufs=3)
            for ih in range(2):
                for c in range(DC):
                    nc.tensor.matmul(
                        p_t[:, ih * 512:(ih + 1) * 512],
                        w4[:, c, :],
                        xt[c][:, ih * 512:(ih + 1) * 512],
                        start=(c == 0), stop=(c == DC - 1),
                    )
            if drain == "scalar":
                nc.scalar.copy(dst[mb][:], p_t[:])
            else:
                nc.vector.tensor_copy(dst[mb][:], p_t[:])

        emit_qk_proj("q", qt, 0)
        emit_qk_proj("k", kt, 0)
        # queue the remaining q/k quarters (slots recycle as projections run)
        for mb in range(1, IC):
            w4q[("q", mb)] = load_qk_cast("q", mb, load_qk_dma("q", wq_d, mb))
            w4q[("k", mb)] = load_qk_cast("k", mb, load_qk_dma("k", wk_d, mb))

        for jp in range(4):
            p_t = ps_pool.tile([P, N], FP32, tag="big", name=f"pv{jp}", bufs=3)
            for half in range(2):
                jb = 2 * jp + half
                for c in range(DC):
                    nc.tensor.matmul(
                        p_t[:, half * 512:(half + 1) * 512],
                        xt[c][:, jb * P:(jb + 1) * P],
                        wv_b[:, c, :],
                        start=(c == 0), stop=(c == DC - 1),
                    )
                nc.vector.tensor_copy(
                    vts[jb][:, :, 0:DH],
                    p_t[:, half * 512:(half + 1) * 512]
                    .rearrange("p (h d) -> p h d", h=H),
                )

        # Wo quarters: natural layout [128, 1024] rows mbi*128..  (fp32 DMA
        # staging, cast to bf16 on the vector engine which has slack here)
        wo4 = []
        for mbi in range(IC):
            stg = stg_pool.tile([P, D], FP32, tag="stw", name=f"stw{mbi}")
            nc.scalar.dma_start(
                out=stg[:],
                in_=wo_d.ap()[mbi * P:(mbi + 1) * P, :],
            )
            w4 = w4_pool.tile([P, D], BF16, tag="w4o", name=f"w4o{mbi}")
            nc.vector.tensor_copy(w4[:], stg[:])
            wo4.append(w4)

        # ---------------- attention: software-pipelined head loop ----------
        # PE filler per head: q/k projections for later head-pairs; head 7
        # runs output-projection PASS 1 instead.
        filler = {0: ("q", 1), 1: ("k", 1), 2: ("q", 2), 3: ("k", 2),
                  4: ("q", 3), 5: ("k", 3)}
        us_tiles = {}
        rrec_tiles = {}
        etp_tiles = {}
        v2_tiles = {}

        def emit_recip(g):
            rrec = smb_pool.tile([1, N], FP32, tag="rrec", name=f"rrec{g}")
            nc.vector.reciprocal(rrec[:], us_tiles[g][DH:DH + 1, :])
            rrec_tiles[g] = rrec

        def emit_finish(g):
            gmb, goff = g // 2, (g % 2) * DH
            bc_sb = sm_pool.tile([DH, N], FP32, tag="bc_sb", name=f"bcs{g}")
            nc.gpsimd.partition_broadcast(bc_sb[:], rrec_tiles[g][:])
            nc.vector.tensor_mul(
                ot[gmb][goff:goff + DH, :],
                us_tiles[g][0:DH, :],
                bc_sb[:],
            )

        def emit_S_block(h, jb, c_all, v2f, etp):
            mb, off = h // 2, (h % 2) * DH
            kth = kt[mb][off:off + DH, :]
            qth = qt[mb][off:off + DH, :]
            # S^T block [128 j, 1024 i] in PSUM (2 banks)
            p_s = ps_pool.tile([P, N], FP32, tag="big", name=f"s{h}_{jb}", bufs=3)
            for ih in range(2):
                nc.tensor.matmul(
                    p_s[:, ih * 512:(ih + 1) * 512],
                    kth[:, jb * P:(jb + 1) * P],
                    qth[:, ih * 512:(ih + 1) * 512],
                    start=True, stop=True,
                )
            # fused exp + softmax-denominator C[j]; rounds to bf16
            nc.scalar.activation(
                etp[jb // 2][:, jb % 2, :], p_s[:],
                mybir.ActivationFunctionType.Exp,
                scale=SCALE, accum_out=c_all[:, jb:jb + 1],
            )
            # V' = (V | ones)/C[j] on GPSIMD -> (V/C | 1/C), fp32
            nc.gpsimd.normalize_recip(
                v2f[:, jb, :],
                vts[jb][:, h, :],
                c_all[:, jb:jb + 1],
            )

        def emit_U(g):
            # U^T[d, i] accumulated over j ; row DH = R[i]
            etp, v2all = etp_tiles[g], v2_tiles[g]
            p_us = []
            for ih in range(2):
                p_u = ps_pool.tile([DH + 1, 512], FP32, tag="u",
                                   name=f"u{g}_{ih}", bufs=2)
                for jb in range(NB):
                    nc.tensor.matmul(
                        p_u[:],
                        v2all[:, jb, :],
                        etp[jb // 2][:, jb % 2, ih * 512:(ih + 1) * 512],
                        start=(jb == 0), stop=(jb == NB - 1),
                    )
                p_us.append(p_u)
            us = usb_pool.tile([DH + 1, N], FP32, tag="usb", name=f"usb{g}")
            for ih in range(2):
                nc.vector.tensor_copy(
                    us[:, ih * 512:(ih + 1) * 512], p_us[ih][:]
                )
            us_tiles[g] = us

        def emit_opass1(ibs, drain):
            # output projection over heads 0-5 (+bias); drains to bf16 y1
            for ib in ibs:
                p_y = ps_pool.tile([P, N], FP32, tag="big", name=f"py{ib}", bufs=3)
                for db in range(2):
                    nc.tensor.matmul(
                        p_y[:, db * 512:(db + 1) * 512],
                        ones_b[:], bo_r[:, db, :],
                        start=True, stop=False,
                    )
                    for mbi in range(IC - 1):
                        nc.tensor.matmul(
                            p_y[:, db * 512:(db + 1) * 512],
                            ot[mbi][:, ib * P:(ib + 1) * P],
                            wo4[mbi][:, db * 512:(db + 1) * 512],
                            start=False, stop=(mbi == IC - 2),
                        )
                if drain == "scalar":
                    nc.scalar.copy(y1[ib][:], p_y[:])
                else:
                    nc.vector.tensor_copy(y1[ib][:], p_y[:])

        for h in range(H):
            c_all = sm_pool.tile([P, NB], FP32, tag="c_all", name=f"ca{h}")
            v2f = sm_pool.tile([P, NB, DH + 1], FP32, tag="v2f", name=f"v2f{h}")
            v2all = sm_pool.tile([P, NB, DH + 1], BF16, tag="v2", name=f"v2_{h}")
            etp = [
                et_pool.tile([P, 2, N], BF16, tag=f"etp{jp}", name=f"etp{h}_{jp}")
                for jp in range(NB // 2)
            ]
            etp_tiles[h], v2_tiles[h] = etp, v2all

            for jb in range(3):
                emit_S_block(h, jb, c_all, v2f, etp)
            if h >= 1:
                emit_U(h - 1)
            for jb in range(3, NB):
                emit_S_block(h, jb, c_all, v2f, etp)
                if jb == 3:
                    # v2 casts on DVE, emitted BEFORE the reciprocal so the
                    # next head's U matmuls are never queued behind it
                    nc.vector.tensor_copy(v2all[:, 0:4, :], v2f[:, 0:4, :])
            if h == H - 1:
                # head 7: finish(5) sandwiched mid-head (its reciprocal is
                # already done, so the gpsimd broadcast costs only 1.8us
                # here), then pass 1 of the output projection as PE filler,
                # split around U(7) + recip(7).  pass-1 drains on ACT --
                # DVE is running the two tail reciprocals.
                emit_finish(5)
                nc.vector.tensor_copy(v2all[:, 4:8, :], v2f[:, 4:8, :])
                emit_recip(6)
                emit_opass1(range(0, 4), drain="scalar")
                emit_U(H - 1)
                emit_recip(H - 1)
                emit_opass1(range(4, NB), drain="scalar")
            else:
                nc.vector.tensor_copy(v2all[:, 4:8, :], v2f[:, 4:8, :])
                # filler BEFORE the reciprocal: its DVE drain must not queue
                # behind the 6.5us reciprocal
                if h in filler:
                    key, fmb = filler[h]
                    emit_qk_proj(key, qt if key == "q" else kt, fmb)
                if h >= 1:
                    emit_recip(h - 1)
                # finish(h-2) at the END of the head: its gpsimd broadcast
                # waits on recip(h-2), and queueing it before the head's
                # normalize_recips was blocking them (and the v2 casts, and
                # the next head's U matmuls) behind that wait
                if h >= 2:
                    emit_finish(h - 2)

        emit_finish(H - 2)
        emit_finish(H - 1)

        # ---------------- output projection pass 2 (heads 6,7) ------------
        for ib in range(NB):
            p_y = ps_pool.tile([P, N], FP32, tag="big", name=f"py2_{ib}", bufs=3)
            for db in range(2):
                nc.tensor.matmul(
                    p_y[:, db * 512:(db + 1) * 512],
                    ot[IC - 1][:, ib * P:(ib + 1) * P],
                    wo4[IC - 1][:, db * 512:(db + 1) * 512],
                    start=True, stop=True,
                )
            y_t = y_pool.tile([P, N], FP32, tag="y", name=f"y{ib}")
            nc.vector.tensor_add(y_t[:], p_y[:], y1[ib][:])
            nc.sync.dma_start(
                out=y_d.ap()[ib * P:(ib + 1) * P, :],
                in_=y_t[:],
            )

        for p in (ps_pool, y_pool, y1_pool, usb_pool, smb_pool, sm_pool,
                  et_pool, xb_pool, xn_pool, stg_pool, w4_pool, wv_pool,
                  xt_pool, ot_pool, v_pool, kt_pool, qt_pool, const_pool):
            p.release()

    nc.finalize()
    return nc


def _get_nc():
    global _NC_CACHE
    if _NC_CACHE is None:
        _NC_CACHE = _build_nc()
    return _NC_CACHE


def kernel(x, Wq, Wk, Wv, Wo, bo, _trace=False, **trace_kwargs):
    x = np.ascontiguousarray(np.asarray(x, dtype=np.float32))
    Wq = np.ascontiguousarray(np.asarray(Wq, dtype=np.float32))
    Wk = np.ascontiguousarray(np.asarray(Wk, dtype=np.float32))
    Wv = np.ascontiguousarray(np.asarray(Wv, dtype=np.float32))
    Wo = np.ascontiguousarray(np.asarray(Wo, dtype=np.float32))
    bo = np.ascontiguousarray(np.asarray(bo, dtype=np.float32))

    nc = _get_nc()
    in_maps = [
        {"x": x[c], "Wq": Wq, "Wk": Wk, "Wv": Wv, "Wo": Wo, "bo": bo}
        for c in range(NCORES)
    ]
    res = run_bass_kernel_spmd(
        nc, in_maps, core_ids=list(range(NCORES)), trace=_trace, **trace_kwargs
    )
    out = np.stack([res.results[c]["y"] for c in range(NCORES)], axis=0)
    if _trace:
        return out.astype(np.float32), res
    return out.astype(np.float32)


if __name__ == "__main__":
    rng = np.random.default_rng(0)
    xs = rng.standard_normal((B, N, D), dtype=np.float32)
    wq = rng.standard_normal((D, INNER), dtype=np.float32) * D ** -0.5
    wk = rng.standard_normal((D, INNER), dtype=np.float32) * D ** -0.5
    wv = rng.standard_normal((D, INNER), dtype=np.float32) * D ** -0.5
    wo = rng.standard_normal((INNER, D), dtype=np.float32) * INNER ** -0.5
    bz = np.zeros((D,), dtype=np.float32)
    y = kernel(xs, wq, wk, wv, wo, bz)
    print("ran ok", y.shape, float(np.abs(y).mean()))
